# revision 7
# baseline (speedup 1.0000x reference)
"""Trainium2 Bass kernel for nn_Angles2Backbone.

Full inputs:  input [1024, 3, 512] f32 (phi/psi/omega dihedrals), angles_length [1024] i64.
Full output:  [1024, 4608] f32 backbone coords (N, CA, C per residue, xyz interleaved).

Strategy: pure data parallelism — 128 protein chains per NeuronCore (batch on the
partition axis), 512 residues on the free axis.  Per chain the NeRF transform chain
is composed per-residue into a single rotor quaternion Q_r plus intra-residue atom
offsets u0/u1/u2; a blocked inclusive quaternion scan over residues gives cumulative
frames; bond offsets are rotated into the global frame and residue displacements are
prefix-summed with the native tensor_tensor_scan recurrence.

Math (validated in float64 against the reference formula):
  atom rotor     = qz(a/2) * qx(b_k/2)
  residue rotor  Q = qz(phi/2) qx(b0/2) qz(psi/2) qx(b1/2) qz(omega/2) qx(b2/2)
  q2 = qz(phi)qx(b0)qz(psi) = (cb0*cos(S), sb0*cos(D), sb0*sin(D), cb0*sin(S)),
       S = (phi+psi)/2, D = (phi-psi)/2      (angle-sum identity)
  u0 = tN;  u1 = u0 + Rz(phi)Rx(b0) tCA;  u2 = u1 + Rz(phi)Rx(b0)Rz(psi)Rx(b1) tC
  coords(r,k) = B_{r-1} + R(Qcum_{r-1}) u_k ;  B = cumsum of R(Qcum_{r-1}) u2
"""

import math

import numpy as np

N_CORES = 8
B_FULL = 1024
L = 512  # residues per chain
CB = B_FULL // N_CORES  # chains per core = 128 partitions
NB = 64  # number of scan blocks (residue level)
G = L // NB  # serial scan block size = 8

# geometry constants
R_CA_C = 1.525
R_C_N = 1.330
R_N_CA = 1.460
CA_C_N = math.pi - 2.1186
C_N_CA = math.pi - 1.9391
N_CA_C = math.pi - 2.061

B_K = [C_N_CA, N_CA_C, CA_C_N]  # bend angle per atom slot (N, CA, C)
R_KC = [R_C_N, R_N_CA, R_CA_C]  # bond length per atom slot

HALF_PI = math.pi / 2.0

# quaternion product index pairs: m[k] = L[a_k] * R[b_k]
_QPAIRS = [
    (0, 0), (1, 1), (2, 2), (3, 3),  # w terms
    (0, 1), (1, 0), (2, 3), (3, 2),  # x terms
    (0, 2), (1, 3), (2, 0), (3, 1),  # y terms
    (0, 3), (1, 2), (2, 1), (3, 0),  # z terms
]


def _body(ctx, tc, out_ap, inp_ap, lens_ap):
    import concourse.mybir as mybir

    nc = tc.nc
    dt = mybir.dt.float32
    Alu = mybir.AluOpType
    Act = mybir.ActivationFunctionType

    cb1h, sb1h = math.cos(B_K[1] / 2), math.sin(B_K[1] / 2)
    cb2h, sb2h = math.cos(B_K[2] / 2), math.sin(B_K[2] / 2)
    cb0h, sb0h = math.cos(B_K[0] / 2), math.sin(B_K[0] / 2)
    cb0f, sb0f = math.cos(B_K[0]), math.sin(B_K[0])
    cb1f, sb1f = math.cos(B_K[1]), math.sin(B_K[1])

    # round-robin engine picker for tensor-tensor class ops (DVE faster than Pool)
    _rr = [0]

    def E():
        _rr[0] += 1
        return nc.vector if _rr[0] % 3 else nc.gpsimd

    def tt(o, a, b, op):
        E().tensor_tensor(out=o, in0=a, in1=b, op=op)

    def stt(o, in0, scalar, in1, op0, op1):
        # TensorScalarPtr ops are DVE-only on TRN2 (Pool rejects them)
        nc.vector.scalar_tensor_tensor(out=o, in0=in0, scalar=scalar, in1=in1,
                                       op0=op0, op1=op1)

    def ts(o, a, s1, s2=None):
        # o = a * s1 (+ s2)  on ACT (frees DVE/Pool for tensor-tensor work)
        nc.scalar.activation(o, a, Act.Identity,
                             bias=(0.0 if s2 is None else cval(s2)), scale=s1)

    def ts_v(o, a, s1):
        # o = a * s1 (TensorScalarPtr is DVE-only on TRN2)
        nc.vector.tensor_scalar(out=o, in0=a, scalar1=s1, scalar2=None, op0=Alu.mult)

    persist = ctx.enter_context(tc.tile_pool(name="persist", bufs=1))
    u = [persist.tile([CB, 3 * L], dt, name=f"u_{d}") for d in range(3)]
    w = [persist.tile([CB, 3 * L], dt, name=f"w_{d}") for d in range(3)]
    Qc = [persist.tile([CB, L + 1], dt, name=f"Qc_{c}") for c in range(4)]
    Binc = [persist.tile([CB, L + 1], dt, name=f"Binc_{d}") for d in range(3)]
    out_sb = persist.tile([CB, 9 * L], dt, name="out_sb")
    ones = persist.tile([CB, L], dt, name="ones")
    mask = persist.tile([CB, L], dt, name="mask")
    lens_sb = persist.tile([CB, 1], dt, name="lens_sb")

    nc.gpsimd.memset(ones[:], 1.0)
    nc.sync.dma_start(lens_sb[:], lens_ap)

    # constant [CB,1] planes for activation biases
    _consts = {}

    def cval(v):
        if v not in _consts:
            t = persist.tile([CB, 1], dt, name=f"cval_{len(_consts)}")
            nc.gpsimd.memset(t[:], v)
            _consts[v] = t[:]
        return _consts[v]

    # interleaved views: column 3r + k (k = atom slot within residue)
    uv = [u[d][:].rearrange("p (r k) -> p r k", k=3) for d in range(3)]
    wv = [w[d][:].rearrange("p (r k) -> p r k", k=3) for d in range(3)]
    u0 = [uv[d][:, :, 0] for d in range(3)]
    u1 = [uv[d][:, :, 1] for d in range(3)]
    u2 = [uv[d][:, :, 2] for d in range(3)]

    Qin = [Qc[c][:][:, 1:L + 1] for c in range(4)]
    Qex = [Qc[c][:][:, 0:L] for c in range(4)]

    # ---------------- Phase A: load + trig ----------------
    phase_b = tc.tile_pool(name="phase_b", bufs=1)
    pb = phase_b.__enter__()
    dih = pb.tile([CB, 3, L], dt, name="dih")
    nc.sync.dma_start(dih[:], inp_ap)
    phi = dih[:][:, 0, :]
    psi = dih[:][:, 1, :]
    omg = dih[:][:, 2, :]

    def bplane(name):
        return pb.tile([CB, L], dt, name=name)

    # ScalarE Sin domain is [-pi, pi]; cosines use cos(y) = 1 - 2 sin^2(y/2).
    cf = [bplane(f"cf{i}") for i in range(3)]  # cos(full angle)
    sf = [bplane(f"sf{i}") for i in range(3)]
    sq = bplane("sqtmp")
    sOh = bplane("sOh")
    for i, ang in enumerate((phi, psi, omg)):
        nc.scalar.activation(sf[i][:], ang, Act.Sin, bias=0.0, scale=1.0)
        half = sOh if i == 2 else sq  # sin(omega/2) reused by the rotor build
        nc.scalar.activation(half[:], ang, Act.Sin, bias=0.0, scale=0.5)
        tt(cf[i][:], half[:], half[:], Alu.mult)
        ts(cf[i][:], cf[i][:], -2.0, 1.0)

    ssum = bplane("ssum")
    sdif = bplane("sdif")
    tt(ssum[:], phi, psi, Alu.add)
    tt(sdif[:], phi, psi, Alu.subtract)

    cS = bplane("cS"); sS = bplane("sS")
    cD = bplane("cD"); sD = bplane("sD")
    cOh = bplane("cOh")
    nc.scalar.activation(sS[:], ssum[:], Act.Sin, bias=0.0, scale=0.5)
    nc.scalar.activation(sD[:], sdif[:], Act.Sin, bias=0.0, scale=0.5)
    nc.scalar.activation(cS[:], ssum[:], Act.Sin, bias=0.0, scale=0.25)
    tt(cS[:], cS[:], cS[:], Alu.mult)
    ts(cS[:], cS[:], -2.0, 1.0)
    nc.scalar.activation(cD[:], sdif[:], Act.Sin, bias=0.0, scale=0.25)
    tt(cD[:], cD[:], cD[:], Alu.mult)
    ts(cD[:], cD[:], -2.0, 1.0)
    nc.scalar.activation(cOh[:], omg, Act.Sin, bias=cval(HALF_PI), scale=0.5)

    # ---------------- Phase B: residue rotor Q + intra-residue u vectors ------
    q2 = [bplane(f"q2_{c}") for c in range(4)]
    ts(q2[0][:], cS[:], cb0h)
    ts(q2[1][:], cD[:], sb0h)
    ts(q2[2][:], sD[:], sb0h)
    ts(q2[3][:], sS[:], cb0h)

    # q3 = q2 * qx(b1h)
    q3 = [bplane(f"q3_{c}") for c in range(4)]
    qt = [bplane(f"qt_{c}") for c in range(4)]
    ts(qt[0][:], q2[1][:], sb1h)
    stt(q3[0][:], q2[0][:], cb1h, qt[0][:], Alu.mult, Alu.subtract)
    ts(qt[1][:], q2[0][:], sb1h)
    stt(q3[1][:], q2[1][:], cb1h, qt[1][:], Alu.mult, Alu.add)
    ts(qt[2][:], q2[3][:], sb1h)
    stt(q3[2][:], q2[2][:], cb1h, qt[2][:], Alu.mult, Alu.add)
    ts(qt[3][:], q2[2][:], sb1h)
    stt(q3[3][:], q2[3][:], cb1h, qt[3][:], Alu.mult, Alu.subtract)

    # q4 = q3 * qz(omega/2)
    q4 = [bplane(f"q4_{c}") for c in range(4)]
    zp = [bplane(f"zp_{c}") for c in range(4)]
    tt(q4[0][:], q3[0][:], cOh[:], Alu.mult)
    tt(zp[0][:], q3[3][:], sOh[:], Alu.mult)
    tt(q4[0][:], q4[0][:], zp[0][:], Alu.subtract)
    tt(q4[1][:], q3[1][:], cOh[:], Alu.mult)
    tt(zp[1][:], q3[2][:], sOh[:], Alu.mult)
    tt(q4[1][:], q4[1][:], zp[1][:], Alu.add)
    tt(q4[2][:], q3[2][:], cOh[:], Alu.mult)
    tt(zp[2][:], q3[1][:], sOh[:], Alu.mult)
    tt(q4[2][:], q4[2][:], zp[2][:], Alu.subtract)
    tt(q4[3][:], q3[3][:], cOh[:], Alu.mult)
    tt(zp[3][:], q3[0][:], sOh[:], Alu.mult)
    tt(q4[3][:], q4[3][:], zp[3][:], Alu.add)

    # Q = q4 * qx(b2h) -> Qin (inclusive-scan storage)
    ts(qt[0][:], q4[1][:], sb2h)
    stt(Qin[0], q4[0][:], cb2h, qt[0][:], Alu.mult, Alu.subtract)
    ts(qt[1][:], q4[0][:], sb2h)
    stt(Qin[1], q4[1][:], cb2h, qt[1][:], Alu.mult, Alu.add)
    ts(qt[2][:], q4[3][:], sb2h)
    stt(Qin[2], q4[2][:], cb2h, qt[2][:], Alu.mult, Alu.add)
    ts(qt[3][:], q4[2][:], sb2h)
    stt(Qin[3], q4[3][:], cb2h, qt[3][:], Alu.mult, Alu.subtract)

    # exclusive-prefix identity at column 0
    nc.gpsimd.memset(Qc[0][:][:, 0:1], 1.0)
    for c in range(1, 4):
        nc.gpsimd.memset(Qc[c][:][:, 0:1], 0.0)

    # --- u vectors ---
    p1 = bplane("p1"); p2 = bplane("p2")
    p3 = bplane("p3"); p4 = bplane("p4")
    tt(p1[:], cf[0][:], cf[1][:], Alu.mult)
    tt(p2[:], sf[0][:], sf[1][:], Alu.mult)
    tt(p3[:], sf[0][:], cf[1][:], Alu.mult)
    tt(p4[:], cf[0][:], sf[1][:], Alu.mult)

    v0 = [bplane(f"v0_{d}") for d in range(3)]
    stt(v0[0][:], p2[:], -cb0f, p1[:], Alu.mult, Alu.add)
    stt(v0[1][:], p4[:], cb0f, p3[:], Alu.mult, Alu.add)
    ts(v0[2][:], sf[1][:], sb0f)

    # u0 = tN = (R0*cphi, R0*sphi, 0); first atom of chain 0-bond fix
    ts_v(u0[0], cf[0][:], R_KC[0])
    ts_v(u0[1], sf[0][:], R_KC[0])
    nc.vector.memset(u0[2], 0.0)
    nc.vector.memset(u[0][:][:, 0:1], 0.0)
    nc.vector.memset(u[1][:][:, 0:1], 0.0)

    # u1 = u0 + R1 * v0
    stt(u1[0], v0[0][:], R_KC[1], u0[0], Alu.mult, Alu.add)
    stt(u1[1], v0[1][:], R_KC[1], u0[1], Alu.mult, Alu.add)
    ts_v(u1[2], v0[2][:], R_KC[1])

    # col1 of Rz(phi)Rx(b0)Rz(psi)Rx(b1)
    c1x = bplane("c1x"); c1y = bplane("c1y"); c1z = bplane("c1z")
    ts(c1x[:], sf[0][:], sb0f * sb1f)
    stt(c1x[:], p3[:], -cb0f * cb1f, c1x[:], Alu.mult, Alu.add)
    stt(c1x[:], p4[:], -cb1f, c1x[:], Alu.mult, Alu.add)
    ts(c1y[:], cf[0][:], -sb0f * sb1f)
    stt(c1y[:], p1[:], cb0f * cb1f, c1y[:], Alu.mult, Alu.add)
    stt(c1y[:], p2[:], -cb1f, c1y[:], Alu.mult, Alu.add)
    ts(c1z[:], cf[1][:], sb0f * cb1f, cb0f * sb1f)

    # u2 = u1 + R2*(comega*v0 + somega*col1)
    for d, c1 in enumerate((c1x, c1y, c1z)):
        qa = qt[d]
        tt(qa[:], cf[2][:], v0[d][:], Alu.mult)
        tt(zp[d][:], sf[2][:], c1[:], Alu.mult)
        tt(qa[:], qa[:], zp[d][:], Alu.add)
        stt(u2[d], qa[:], R_KC[2], u1[d], Alu.mult, Alu.add)

    # mask = (r < length) as 0/1
    iota = bplane("iota")
    nc.gpsimd.iota(iota[:], pattern=[[1, L]], base=0, channel_multiplier=0,
                   allow_small_or_imprecise_dtypes=True)
    nc.vector.tensor_scalar(out=mask[:], in0=iota[:], scalar1=lens_sb[:],
                            scalar2=None, op0=Alu.is_lt)

    phase_b.__exit__(None, None, None)

    # ---------------- Phase C: blocked inclusive quaternion scan --------------
    scan_pool = ctx.enter_context(tc.tile_pool(name="scan", bufs=1))
    tmp = [scan_pool.tile([CB, L], dt, name=f"tmp_{i}") for i in range(16)]

    def tview(tile_, ref_ap):
        """view of tile_ with the same free shape as ref_ap"""
        shp = ref_ap.shape
        if len(shp) == 2:
            return tile_[:][:, 0:shp[1]]
        return (tile_[:][:, 0:shp[1] * shp[2]]
                .rearrange("p (a b) -> p a b", b=shp[2]))

    def qcombine(Lap, Rap, Oap):
        """O = L ⊗ R (quaternion product) on 4-plane lists of equal-shape APs."""
        mv = []
        for k, (a, b) in enumerate(_QPAIRS):
            dst = tview(tmp[k], Rap[0])
            tt(dst, Lap[a], Rap[b], Alu.mult)
            mv.append(dst)
        tt(mv[0], mv[0], mv[1], Alu.subtract)
        tt(mv[2], mv[2], mv[3], Alu.add)
        tt(Oap[0], mv[0], mv[2], Alu.subtract)
        tt(mv[4], mv[4], mv[5], Alu.add)
        tt(mv[6], mv[6], mv[7], Alu.subtract)
        tt(Oap[1], mv[4], mv[6], Alu.add)
        tt(mv[8], mv[8], mv[9], Alu.subtract)
        tt(mv[10], mv[10], mv[11], Alu.add)
        tt(Oap[2], mv[8], mv[10], Alu.add)
        tt(mv[12], mv[12], mv[13], Alu.add)
        tt(mv[15], mv[15], mv[14], Alu.subtract)
        tt(Oap[3], mv[12], mv[15], Alu.add)

    # L1: serial scan within blocks of G (slot stride G)
    for i in range(1, G):
        Lap = [Qin[c][:, (i - 1)::G] for c in range(4)]
        Rap = [Qin[c][:, i::G] for c in range(4)]
        qcombine(Lap, Rap, Rap)

    # L2: Hillis-Steele doubling over the NB block aggregates (slot G-1)
    s = 1
    while s < NB:
        agg = [Qin[c][:, (G - 1)::G] for c in range(4)]
        Lap = [a[:, 0:NB - s] for a in agg]
        Rap = [a[:, s:NB] for a in agg]
        qcombine(Lap, Rap, Rap)
        s *= 2

    # Apply block prefixes onto positions 0..G-2 of blocks 1..NB-1
    tgt = [Qin[c][:, G:L].rearrange("p (b g) -> p b g", g=G)[:, :, 0:G - 1]
           for c in range(4)]
    pre = [Qin[c][:, (G - 1)::G][:, 0:NB - 1].unsqueeze(2)
           .broadcast_to((CB, NB - 1, G - 1)) for c in range(4)]
    qcombine(pre, tgt, tgt)

    # ---------------- Phase D: rotate u vectors by exclusive prefix -----------
    rot_pool = ctx.enter_context(tc.tile_pool(name="rot", bufs=1))

    def rplane(name):
        return rot_pool.tile([CB, L], dt, name=name)

    xx = rplane("xx"); yy = rplane("yy"); zz = rplane("zz")
    xy = rplane("xy"); xz = rplane("xz"); yz = rplane("yz")
    wx = rplane("wx"); wy = rplane("wy"); wz = rplane("wz")
    qw, qx, qy, qz = Qex
    tt(xx[:], qx, qx, Alu.mult)
    tt(yy[:], qy, qy, Alu.mult)
    tt(zz[:], qz, qz, Alu.mult)
    tt(xy[:], qx, qy, Alu.mult)
    tt(xz[:], qx, qz, Alu.mult)
    tt(yz[:], qy, qz, Alu.mult)
    tt(wx[:], qw, qx, Alu.mult)
    tt(wy[:], qw, qy, Alu.mult)
    tt(wz[:], qw, qz, Alu.mult)
    S1 = rplane("S1"); S2 = rplane("S2"); S3 = rplane("S3")
    A1 = rplane("A1"); D1 = rplane("D1")
    A2 = rplane("A2"); D2 = rplane("D2")
    A3 = rplane("A3"); D3 = rplane("D3")
    tt(S1[:], yy[:], zz[:], Alu.add)
    tt(S2[:], xx[:], zz[:], Alu.add)
    tt(S3[:], xx[:], yy[:], Alu.add)
    tt(A1[:], xy[:], wz[:], Alu.add)
    tt(D1[:], xy[:], wz[:], Alu.subtract)
    tt(A2[:], xz[:], wy[:], Alu.add)
    tt(D2[:], xz[:], wy[:], Alu.subtract)
    tt(A3[:], yz[:], wx[:], Alu.add)
    tt(D3[:], yz[:], wx[:], Alu.subtract)

    wt = [rot_pool.tile([CB, 3 * L], dt, name=f"wt_{i}") for i in range(6)]

    def bc(plane):  # broadcast [CB, L] -> [CB, L, 3]
        return plane[:].unsqueeze(2).broadcast_to((CB, L, 3))

    # w_x = ux + 2*(uy*D1 + uz*A2 - ux*S1)
    rows = [
        (0, (1, D1), (2, A2), (0, S1), wt[0], wt[1]),
        (1, (0, A1), (2, D3), (1, S2), wt[2], wt[3]),
        (2, (0, D2), (1, A3), (2, S3), wt[4], wt[5]),
    ]
    for d, (ia, Pa), (ib, Pb), (ic, Pc), t1, t2 in rows:
        t1v = tview(t1, uv[0])
        t2v = tview(t2, uv[0])
        tt(t1v, uv[ia], bc(Pa), Alu.mult)
        tt(t2v, uv[ib], bc(Pb), Alu.mult)
        tt(t1v, t1v, t2v, Alu.add)
        tt(t2v, uv[ic], bc(Pc), Alu.mult)
        tt(t1v, t1v, t2v, Alu.subtract)
        stt(wv[d], t1v, 2.0, uv[d], Alu.mult, Alu.add)

    # ---------------- Phase E: boundary cumsum + final coords -----------------
    for d in range(3):
        nc.vector.memset(Binc[d][:][:, 0:1], 0.0)
        nc.vector.tensor_tensor_scan(
            out=Binc[d][:][:, 1:L + 1],
            data0=ones[:],
            data1=w[d][:][:, 2::3],
            initial=0.0,
            op0=Alu.mult,
            op1=Alu.add,
        )

    outv = out_sb[:].rearrange("p (r k d) -> p r k d", k=3, d=3)
    for d in range(3):
        Bex = Binc[d][:][:, 0:L].unsqueeze(2).broadcast_to((CB, L, 3))
        ta = tview(wt[2 * d], wv[d])
        tt(ta, wv[d], Bex, Alu.add)
        tt(outv[:, :, :, d], ta, bc(mask), Alu.mult)

    nc.sync.dma_start(out_ap, out_sb[:])


_CACHE = {}


def _build():
    from contextlib import ExitStack

    import concourse.bacc as bacc
    import concourse.mybir as mybir
    import concourse.tile as tile

    nc = bacc.Bacc("TRN2", target_bir_lowering=False, debug=False,
                   num_devices=N_CORES)
    inp = nc.dram_tensor("input", [CB, 3, L], mybir.dt.float32,
                         kind="ExternalInput").ap()
    lens = nc.dram_tensor("lens", [CB, 1], mybir.dt.float32,
                          kind="ExternalInput").ap()
    out = nc.dram_tensor("out", [CB, 9 * L], mybir.dt.float32,
                         kind="ExternalOutput").ap()
    with tile.TileContext(nc) as tc_ctx, ExitStack() as ctx:
        _body(ctx, tc_ctx, out, inp, lens)
    nc.compile()
    return nc


def get_nc():
    if "nc" not in _CACHE:
        _CACHE["nc"] = _build()
    return _CACHE["nc"]


def make_in_maps(input, angles_length):
    inp = np.ascontiguousarray(np.asarray(input, dtype=np.float32))
    lens = np.asarray(angles_length).astype(np.float32).reshape(B_FULL, 1)
    in_maps = []
    for i in range(N_CORES):
        sl = slice(i * CB, (i + 1) * CB)
        in_maps.append({
            "input": np.ascontiguousarray(inp[sl]),
            "lens": np.ascontiguousarray(lens[sl]),
        })
    return in_maps


def kernel(input, angles_length):
    from concourse.bass_utils import run_bass_kernel_spmd

    nc = get_nc()
    in_maps = make_in_maps(input, angles_length)
    res = run_bass_kernel_spmd(nc, in_maps, core_ids=list(range(N_CORES)))
    outs = [res.results[i]["out"] for i in range(N_CORES)]
    return np.concatenate(outs, axis=0).astype(np.float32)


# revision 8
# speedup vs baseline: 1.0135x; 1.0135x over previous
"""Trainium2 Bass kernel for nn_Angles2Backbone.

Full inputs:  input [1024, 3, 512] f32 (phi/psi/omega dihedrals), angles_length [1024] i64.
Full output:  [1024, 4608] f32 backbone coords (N, CA, C per residue, xyz interleaved).

Strategy: pure data parallelism — 128 protein chains per NeuronCore (batch on the
partition axis), 512 residues on the free axis.  Per chain the NeRF transform chain
is composed per-residue into a single rotor quaternion Q_r plus intra-residue atom
offsets u0/u1/u2; a blocked inclusive quaternion scan over residues gives cumulative
frames; bond offsets are rotated into the global frame and residue displacements are
prefix-summed with the native tensor_tensor_scan recurrence.

Precision split: the residue rotor build and the scan run in f32 (errors there
accumulate across the chain); the per-residue local quantities (u vectors, frame
application, displacements) run in bf16 (leaf errors, ~0.4% of Angstrom-scale
offsets, do not accumulate) which unlocks the DVE 2x perf mode.

Math (validated in float64 against the reference formula):
  atom rotor     = qz(a/2) * qx(b_k/2)
  residue rotor  Q = qz(phi/2) qx(b0/2) qz(psi/2) qx(b1/2) qz(omega/2) qx(b2/2)
  q2 = qz(phi)qx(b0)qz(psi) = (cb0*cos(S), sb0*cos(D), sb0*sin(D), cb0*sin(S)),
       S = (phi+psi)/2, D = (phi-psi)/2      (angle-sum identity)
  u0 = tN;  u1 = u0 + Rz(phi)Rx(b0) tCA;  u2 = u1 + Rz(phi)Rx(b0)Rz(psi)Rx(b1) tC
  coords(r,k) = B_{r-1} + R(Qcum_{r-1}) u_k ;  B = cumsum of R(Qcum_{r-1}) u2
"""

import math

import numpy as np

N_CORES = 8
B_FULL = 1024
L = 512  # residues per chain
CB = B_FULL // N_CORES  # chains per core = 128 partitions
NB = 64  # number of scan blocks (residue level)
G = L // NB  # serial scan block size = 8

# geometry constants
R_CA_C = 1.525
R_C_N = 1.330
R_N_CA = 1.460
CA_C_N = math.pi - 2.1186
C_N_CA = math.pi - 1.9391
N_CA_C = math.pi - 2.061

B_K = [C_N_CA, N_CA_C, CA_C_N]  # bend angle per atom slot (N, CA, C)
R_KC = [R_C_N, R_N_CA, R_CA_C]  # bond length per atom slot

HALF_PI = math.pi / 2.0

# quaternion product index pairs grouped by output component
_QPAIRS = [
    (0, 0), (1, 1), (2, 2), (3, 3),  # w terms
    (0, 1), (1, 0), (2, 3), (3, 2),  # x terms
    (0, 2), (1, 3), (2, 0), (3, 1),  # y terms
    (0, 3), (1, 2), (2, 1), (3, 0),  # z terms
]


def _body(ctx, tc, out_ap, inp_ap, lens_ap):
    import concourse.mybir as mybir

    nc = tc.nc
    f32 = mybir.dt.float32
    bf16 = mybir.dt.bfloat16
    Alu = mybir.AluOpType
    Act = mybir.ActivationFunctionType

    cb0h, sb0h = math.cos(B_K[0] / 2), math.sin(B_K[0] / 2)
    cb1h, sb1h = math.cos(B_K[1] / 2), math.sin(B_K[1] / 2)
    cb2h, sb2h = math.cos(B_K[2] / 2), math.sin(B_K[2] / 2)
    cb0f, sb0f = math.cos(B_K[0]), math.sin(B_K[0])
    cb1f, sb1f = math.cos(B_K[1]), math.sin(B_K[1])

    # round-robin engine picker for tensor-tensor ops (DVE ~1.6x faster than Pool)
    _rr = [0]

    def E():
        _rr[0] += 1
        return nc.vector if _rr[0] % 3 else nc.gpsimd

    def tt(o, a, b, op, eng=None):
        (eng or E()).tensor_tensor(out=o, in0=a, in1=b, op=op)

    def stt(o, in0, scalar, in1, op0, op1):
        # TensorScalarPtr ops are DVE-only on TRN2 (Pool rejects them)
        nc.vector.scalar_tensor_tensor(out=o, in0=in0, scalar=scalar, in1=in1,
                                       op0=op0, op1=op1)

    def ts(o, a, s1, s2=None):
        # o = a * s1 (+ s2)  on ACT (frees DVE/Pool for tensor-tensor work)
        nc.scalar.activation(o, a, Act.Identity,
                             bias=(0.0 if s2 is None else cval(s2)), scale=s1)

    def ts_v(o, a, s1):
        nc.vector.tensor_scalar(out=o, in0=a, scalar1=s1, scalar2=None,
                                op0=Alu.mult)

    def acopy(o, a):
        nc.scalar.activation(o, a, Act.Copy, bias=0.0, scale=1.0)

    persist = ctx.enter_context(tc.tile_pool(name="persist", bufs=1))
    Qc = [persist.tile([CB, L + 1], f32, name=f"Qc_{c}") for c in range(4)]
    Binc = [persist.tile([CB, L + 1], f32, name=f"Binc_{d}") for d in range(3)]
    # u vectors per atom slot (bf16, contiguous); u0 has no z component
    u0 = [persist.tile([CB, L], bf16, name=f"u0_{d}") for d in range(2)]
    u1 = [persist.tile([CB, L], bf16, name=f"u1_{d}") for d in range(3)]
    u2 = [persist.tile([CB, L], bf16, name=f"u2_{d}") for d in range(3)]
    w0 = [persist.tile([CB, L], bf16, name=f"w0_{d}") for d in range(3)]
    w1 = [persist.tile([CB, L], bf16, name=f"w1_{d}") for d in range(3)]
    w2 = [persist.tile([CB, L], bf16, name=f"w2_{d}") for d in range(3)]
    out_sb = persist.tile([CB, 9 * L], f32, name="out_sb")
    ones = persist.tile([CB, L], f32, name="ones")
    mask = persist.tile([CB, L], f32, name="mask")
    lens_sb = persist.tile([CB, 1], f32, name="lens_sb")

    nc.gpsimd.memset(ones[:], 1.0)
    nc.sync.dma_start(lens_sb[:], lens_ap)

    _consts = {}

    def cval(v):
        if v not in _consts:
            t = persist.tile([CB, 1], f32, name=f"cval_{len(_consts)}")
            nc.gpsimd.memset(t[:], v)
            _consts[v] = t[:]
        return _consts[v]

    Qin = [Qc[c][:][:, 1:L + 1] for c in range(4)]
    Qex = [Qc[c][:][:, 0:L] for c in range(4)]

    # ---------------- Phase A: load + trig ----------------
    phase_b = tc.tile_pool(name="phase_b", bufs=1)
    pb = phase_b.__enter__()
    dih = pb.tile([CB, 3, L], f32, name="dih")
    nc.sync.dma_start(dih[:], inp_ap)
    phi = dih[:][:, 0, :]
    psi = dih[:][:, 1, :]
    omg = dih[:][:, 2, :]

    def bplane(name, dt_=f32):
        return pb.tile([CB, L], dt_, name=name)

    # ScalarE Sin domain is [-pi, pi]; cosines use cos(y) = 1 - 2 sin^2(y/2).
    cf = [bplane(f"cf{i}") for i in range(3)]  # cos(full), f32
    sf = [bplane(f"sf{i}") for i in range(3)]  # sin(full), f32
    cfb = [bplane(f"cfb{i}", bf16) for i in range(3)]  # bf16 copies
    sfb = [bplane(f"sfb{i}", bf16) for i in range(3)]
    sq = bplane("sqtmp")
    sOh = bplane("sOh")
    for i, ang in enumerate((phi, psi, omg)):
        nc.scalar.activation(sf[i][:], ang, Act.Sin, bias=0.0, scale=1.0)
        half = sOh if i == 2 else sq  # sin(omega/2) reused by the rotor build
        nc.scalar.activation(half[:], ang, Act.Sin, bias=0.0, scale=0.5)
        tt(cf[i][:], half[:], half[:], Alu.mult)
        ts(cf[i][:], cf[i][:], -2.0, 1.0)
        acopy(cfb[i][:], cf[i][:])
        acopy(sfb[i][:], sf[i][:])

    ssum = bplane("ssum")
    sdif = bplane("sdif")
    tt(ssum[:], phi, psi, Alu.add)
    tt(sdif[:], phi, psi, Alu.subtract)

    cS = bplane("cS"); sS = bplane("sS")
    cD = bplane("cD"); sD = bplane("sD")
    cOh = bplane("cOh")
    nc.scalar.activation(sS[:], ssum[:], Act.Sin, bias=0.0, scale=0.5)
    nc.scalar.activation(sD[:], sdif[:], Act.Sin, bias=0.0, scale=0.5)
    nc.scalar.activation(cS[:], ssum[:], Act.Sin, bias=0.0, scale=0.25)
    tt(cS[:], cS[:], cS[:], Alu.mult)
    ts(cS[:], cS[:], -2.0, 1.0)
    nc.scalar.activation(cD[:], sdif[:], Act.Sin, bias=0.0, scale=0.25)
    tt(cD[:], cD[:], cD[:], Alu.mult)
    ts(cD[:], cD[:], -2.0, 1.0)
    nc.scalar.activation(cOh[:], omg, Act.Sin, bias=cval(HALF_PI), scale=0.5)

    # ---------------- Phase B1: residue rotor Q (f32) -------------------------
    q2 = [bplane(f"q2_{c}") for c in range(4)]
    ts(q2[0][:], cS[:], cb0h)
    ts(q2[1][:], cD[:], sb0h)
    ts(q2[2][:], sD[:], sb0h)
    ts(q2[3][:], sS[:], cb0h)

    # q3 = q2 * qx(b1h)
    q3 = [bplane(f"q3_{c}") for c in range(4)]
    qt = [bplane(f"qt_{c}") for c in range(4)]
    ts(qt[0][:], q2[1][:], sb1h)
    stt(q3[0][:], q2[0][:], cb1h, qt[0][:], Alu.mult, Alu.subtract)
    ts(qt[1][:], q2[0][:], sb1h)
    stt(q3[1][:], q2[1][:], cb1h, qt[1][:], Alu.mult, Alu.add)
    ts(qt[2][:], q2[3][:], sb1h)
    stt(q3[2][:], q2[2][:], cb1h, qt[2][:], Alu.mult, Alu.add)
    ts(qt[3][:], q2[2][:], sb1h)
    stt(q3[3][:], q2[3][:], cb1h, qt[3][:], Alu.mult, Alu.subtract)

    # q4 = q3 * qz(omega/2)
    q4 = [bplane(f"q4_{c}") for c in range(4)]
    zp = [bplane(f"zp_{c}") for c in range(4)]
    tt(q4[0][:], q3[0][:], cOh[:], Alu.mult)
    tt(zp[0][:], q3[3][:], sOh[:], Alu.mult)
    tt(q4[0][:], q4[0][:], zp[0][:], Alu.subtract)
    tt(q4[1][:], q3[1][:], cOh[:], Alu.mult)
    tt(zp[1][:], q3[2][:], sOh[:], Alu.mult)
    tt(q4[1][:], q4[1][:], zp[1][:], Alu.add)
    tt(q4[2][:], q3[2][:], cOh[:], Alu.mult)
    tt(zp[2][:], q3[1][:], sOh[:], Alu.mult)
    tt(q4[2][:], q4[2][:], zp[2][:], Alu.subtract)
    tt(q4[3][:], q3[3][:], cOh[:], Alu.mult)
    tt(zp[3][:], q3[0][:], sOh[:], Alu.mult)
    tt(q4[3][:], q4[3][:], zp[3][:], Alu.add)

    # Q = q4 * qx(b2h) -> Qin
    ts(qt[0][:], q4[1][:], sb2h)
    stt(Qin[0], q4[0][:], cb2h, qt[0][:], Alu.mult, Alu.subtract)
    ts(qt[1][:], q4[0][:], sb2h)
    stt(Qin[1], q4[1][:], cb2h, qt[1][:], Alu.mult, Alu.add)
    ts(qt[2][:], q4[3][:], sb2h)
    stt(Qin[2], q4[2][:], cb2h, qt[2][:], Alu.mult, Alu.add)
    ts(qt[3][:], q4[2][:], sb2h)
    stt(Qin[3], q4[3][:], cb2h, qt[3][:], Alu.mult, Alu.subtract)

    nc.gpsimd.memset(Qc[0][:][:, 0:1], 1.0)
    for c in range(1, 4):
        nc.gpsimd.memset(Qc[c][:][:, 0:1], 0.0)

    # ---------------- Phase B2: u vectors (bf16, contiguous) ------------------
    p1 = bplane("p1", bf16); p2 = bplane("p2", bf16)
    p3 = bplane("p3", bf16); p4 = bplane("p4", bf16)
    tt(p1[:], cfb[0][:], cfb[1][:], Alu.mult)
    tt(p2[:], sfb[0][:], sfb[1][:], Alu.mult)
    tt(p3[:], sfb[0][:], cfb[1][:], Alu.mult)
    tt(p4[:], cfb[0][:], sfb[1][:], Alu.mult)

    v0 = [bplane(f"v0_{d}", bf16) for d in range(3)]
    stt(v0[0][:], p2[:], -cb0f, p1[:], Alu.mult, Alu.add)
    stt(v0[1][:], p4[:], cb0f, p3[:], Alu.mult, Alu.add)
    ts(v0[2][:], sfb[1][:], sb0f)

    ts_v(u0[0][:], cfb[0][:], R_KC[0])
    ts_v(u0[1][:], sfb[0][:], R_KC[0])
    nc.vector.memset(u0[0][:][:, 0:1], 0.0)
    nc.vector.memset(u0[1][:][:, 0:1], 0.0)

    stt(u1[0][:], v0[0][:], R_KC[1], u0[0][:], Alu.mult, Alu.add)
    stt(u1[1][:], v0[1][:], R_KC[1], u0[1][:], Alu.mult, Alu.add)
    ts_v(u1[2][:], v0[2][:], R_KC[1])

    c1x = bplane("c1x", bf16); c1y = bplane("c1y", bf16); c1z = bplane("c1z", bf16)
    ts(c1x[:], sfb[0][:], sb0f * sb1f)
    stt(c1x[:], p3[:], -cb0f * cb1f, c1x[:], Alu.mult, Alu.add)
    stt(c1x[:], p4[:], -cb1f, c1x[:], Alu.mult, Alu.add)
    ts(c1y[:], cfb[0][:], -sb0f * sb1f)
    stt(c1y[:], p1[:], cb0f * cb1f, c1y[:], Alu.mult, Alu.add)
    stt(c1y[:], p2[:], -cb1f, c1y[:], Alu.mult, Alu.add)
    ts(c1z[:], cfb[1][:], sb0f * cb1f, cb0f * sb1f)

    # u2 = u1 + R2*(comega*v0 + somega*col1)
    for d, c1 in enumerate((c1x, c1y, c1z)):
        qa = bplane(f"u2t_{d}", bf16)
        qb = bplane(f"u2s_{d}", bf16)
        tt(qa[:], cfb[2][:], v0[d][:], Alu.mult)
        tt(qb[:], sfb[2][:], c1[:], Alu.mult)
        tt(qa[:], qa[:], qb[:], Alu.add)
        stt(u2[d][:], qa[:], R_KC[2], u1[d][:], Alu.mult, Alu.add)

    # mask = (r < length) as 0/1
    iota = bplane("iota")
    nc.gpsimd.iota(iota[:], pattern=[[1, L]], base=0, channel_multiplier=0,
                   allow_small_or_imprecise_dtypes=True)
    nc.vector.tensor_scalar(out=mask[:], in0=iota[:], scalar1=lens_sb[:],
                            scalar2=None, op0=Alu.is_lt)

    phase_b.__exit__(None, None, None)

    # ---------------- Phase C: blocked inclusive quaternion scan (f32) --------
    scan_pool = ctx.enter_context(tc.tile_pool(name="scan", bufs=1))
    tmp = [scan_pool.tile([CB, L], f32, name=f"tmp_{i}") for i in range(16)]

    def tview(tile_, ref_ap):
        shp = ref_ap.shape
        if len(shp) == 2:
            return tile_[:][:, 0:shp[1]]
        return (tile_[:][:, 0:shp[1] * shp[2]]
                .rearrange("p (a b) -> p a b", b=shp[2]))

    _step = [0]

    def qcombine(Lap, Rap, Oap):
        """O = L ⊗ R quaternion product; components split across DVE/Pool to
        keep each add-chain on one engine (fewer cross-engine semaphores).
        Alternates the 3/1 vs 2/2 component split for balance (DVE faster)."""
        _step[0] += 1
        pool_comps = (3,) if _step[0] % 2 else (2, 3)
        eng = [nc.gpsimd if c in pool_comps else nc.vector for c in range(4)]
        mv = []
        for k, (a, b) in enumerate(_QPAIRS):
            dst = tview(tmp[k], Rap[0])
            tt(dst, Lap[a], Rap[b], Alu.mult, eng=eng[k // 4])
            mv.append(dst)
        specs = [  # (comp, mA, mB, opAB, mC, mD, opCD, opFinal)
            (0, 0, 1, Alu.subtract, 2, 3, Alu.add, Alu.subtract),
            (1, 4, 5, Alu.add, 6, 7, Alu.subtract, Alu.add),
            (2, 8, 9, Alu.subtract, 10, 11, Alu.add, Alu.add),
            (3, 12, 13, Alu.add, 15, 14, Alu.subtract, Alu.add),
        ]
        for comp, a, b, opab, c_, d_, opcd, opf in specs:
            e = eng[comp]
            tt(mv[a], mv[a], mv[b], opab, eng=e)
            tt(mv[c_], mv[c_], mv[d_], opcd, eng=e)
            tt(Oap[comp], mv[a], mv[c_], opf, eng=e)

    # L1: serial scan within blocks of G
    for i in range(1, G):
        Lap = [Qin[c][:, (i - 1)::G] for c in range(4)]
        Rap = [Qin[c][:, i::G] for c in range(4)]
        qcombine(Lap, Rap, Rap)

    # L2: Hillis-Steele doubling over the NB block aggregates (slot G-1)
    s = 1
    while s < NB:
        agg = [Qin[c][:, (G - 1)::G] for c in range(4)]
        Lap = [a[:, 0:NB - s] for a in agg]
        Rap = [a[:, s:NB] for a in agg]
        qcombine(Lap, Rap, Rap)
        s *= 2

    # Apply block prefixes onto positions 0..G-2 of blocks 1..NB-1
    tgt = [Qin[c][:, G:L].rearrange("p (b g) -> p b g", g=G)[:, :, 0:G - 1]
           for c in range(4)]
    pre = [Qin[c][:, (G - 1)::G][:, 0:NB - 1].unsqueeze(2)
           .broadcast_to((CB, NB - 1, G - 1)) for c in range(4)]
    qcombine(pre, tgt, tgt)

    # ---------------- Phase D: rotate u vectors by exclusive prefix (bf16) ----
    rot_pool = ctx.enter_context(tc.tile_pool(name="rot", bufs=1))

    def rplane(name):
        return rot_pool.tile([CB, L], bf16, name=name)

    # bf16 copies of the exclusive prefix
    Qb = [rplane(f"Qb_{c}") for c in range(4)]
    for c in range(4):
        acopy(Qb[c][:], Qex[c])
    qw, qx, qy, qz = [Qb[c][:] for c in range(4)]

    xx = rplane("xx"); yy = rplane("yy"); zz = rplane("zz")
    xy = rplane("xy"); xz = rplane("xz"); yz = rplane("yz")
    wx = rplane("wx"); wy = rplane("wy"); wz = rplane("wz")
    tt(xx[:], qx, qx, Alu.mult)
    tt(yy[:], qy, qy, Alu.mult)
    tt(zz[:], qz, qz, Alu.mult)
    tt(xy[:], qx, qy, Alu.mult)
    tt(xz[:], qx, qz, Alu.mult)
    tt(yz[:], qy, qz, Alu.mult)
    tt(wx[:], qw, qx, Alu.mult)
    tt(wy[:], qw, qy, Alu.mult)
    tt(wz[:], qw, qz, Alu.mult)
    S1 = rplane("S1"); S2 = rplane("S2"); S3 = rplane("S3")
    A1 = rplane("A1"); D1 = rplane("D1")
    A2 = rplane("A2"); D2 = rplane("D2")
    A3 = rplane("A3"); D3 = rplane("D3")
    tt(S1[:], yy[:], zz[:], Alu.add)
    tt(S2[:], xx[:], zz[:], Alu.add)
    tt(S3[:], xx[:], yy[:], Alu.add)
    tt(A1[:], xy[:], wz[:], Alu.add)
    tt(D1[:], xy[:], wz[:], Alu.subtract)
    tt(A2[:], xz[:], wy[:], Alu.add)
    tt(D2[:], xz[:], wy[:], Alu.subtract)
    tt(A3[:], yz[:], wx[:], Alu.add)
    tt(D3[:], yz[:], wx[:], Alu.subtract)

    ta = rplane("rt_a"); tb = rplane("rt_b")

    def rotate(uvec, wvec):
        """wvec = R(Qex) @ uvec; uvec z-component may be None (zero)."""
        ux, uy = uvec[0][:], uvec[1][:]
        uz = uvec[2][:] if len(uvec) > 2 else None
        # rows: (out, term+, termA, term+, termB, term-, termC, base)
        # w_x = ux + 2*(uy*D1 [+ uz*A2] - ux*S1)
        tt(ta[:], uy, D1[:], Alu.mult)
        if uz is not None:
            tt(tb[:], uz, A2[:], Alu.mult)
            tt(ta[:], ta[:], tb[:], Alu.add)
        tt(tb[:], ux, S1[:], Alu.mult)
        tt(ta[:], ta[:], tb[:], Alu.subtract)
        stt(wvec[0][:], ta[:], 2.0, ux, Alu.mult, Alu.add)
        # w_y = uy + 2*(ux*A1 [+ uz*D3] - uy*S2)
        tt(ta[:], ux, A1[:], Alu.mult)
        if uz is not None:
            tt(tb[:], uz, D3[:], Alu.mult)
            tt(ta[:], ta[:], tb[:], Alu.add)
        tt(tb[:], uy, S2[:], Alu.mult)
        tt(ta[:], ta[:], tb[:], Alu.subtract)
        stt(wvec[1][:], ta[:], 2.0, uy, Alu.mult, Alu.add)
        # w_z = [uz +] 2*(ux*D2 + uy*A3 [- uz*S3])
        tt(ta[:], ux, D2[:], Alu.mult)
        tt(tb[:], uy, A3[:], Alu.mult)
        tt(ta[:], ta[:], tb[:], Alu.add)
        if uz is not None:
            tt(tb[:], uz, S3[:], Alu.mult)
            tt(ta[:], ta[:], tb[:], Alu.subtract)
            stt(wvec[2][:], ta[:], 2.0, uz, Alu.mult, Alu.add)
        else:
            ts_v(wvec[2][:], ta[:], 2.0)

    rotate(u0, w0)
    rotate(u1, w1)
    rotate(u2, w2)

    # ---------------- Phase E: boundary cumsum + final coords -----------------
    for d in range(3):
        nc.vector.memset(Binc[d][:][:, 0:1], 0.0)
        nc.vector.tensor_tensor_scan(
            out=Binc[d][:][:, 1:L + 1],
            data0=ones[:],
            data1=w2[d][:],
            initial=0.0,
            op0=Alu.mult,
            op1=Alu.add,
        )

    outv = out_sb[:].rearrange("p (r k d) -> p r k d", k=3, d=3)
    et = [rot_pool.tile([CB, L], f32, name=f"et_{i}") for i in range(3)]
    for k, wk in enumerate((w0, w1, w2)):
        for d in range(3):
            Bex = Binc[d][:][:, 0:L]
            tt(et[d][:], wk[d][:], Bex, Alu.add)
            tt(outv[:, :, k, d], et[d][:], mask[:], Alu.mult)

    nc.sync.dma_start(out_ap, out_sb[:])


_CACHE = {}


def _build():
    from contextlib import ExitStack

    import concourse.bacc as bacc
    import concourse.mybir as mybir
    import concourse.tile as tile

    nc = bacc.Bacc("TRN2", target_bir_lowering=False, debug=False,
                   num_devices=N_CORES)
    inp = nc.dram_tensor("input", [CB, 3, L], mybir.dt.float32,
                         kind="ExternalInput").ap()
    lens = nc.dram_tensor("lens", [CB, 1], mybir.dt.float32,
                          kind="ExternalInput").ap()
    out = nc.dram_tensor("out", [CB, 9 * L], mybir.dt.float32,
                         kind="ExternalOutput").ap()
    with tile.TileContext(nc) as tc_ctx, ExitStack() as ctx:
        _body(ctx, tc_ctx, out, inp, lens)
    nc.compile()
    return nc


def get_nc():
    if "nc" not in _CACHE:
        _CACHE["nc"] = _build()
    return _CACHE["nc"]


def make_in_maps(input, angles_length):
    inp = np.ascontiguousarray(np.asarray(input, dtype=np.float32))
    lens = np.asarray(angles_length).astype(np.float32).reshape(B_FULL, 1)
    in_maps = []
    for i in range(N_CORES):
        sl = slice(i * CB, (i + 1) * CB)
        in_maps.append({
            "input": np.ascontiguousarray(inp[sl]),
            "lens": np.ascontiguousarray(lens[sl]),
        })
    return in_maps


def kernel(input, angles_length):
    from concourse.bass_utils import run_bass_kernel_spmd

    nc = get_nc()
    in_maps = make_in_maps(input, angles_length)
    res = run_bass_kernel_spmd(nc, in_maps, core_ids=list(range(N_CORES)))
    outs = [res.results[i]["out"] for i in range(N_CORES)]
    return np.concatenate(outs, axis=0).astype(np.float32)
